# revision 19
# baseline (speedup 1.0000x reference)
"""BioNorm Trainium2 kernel.

Computes, for x:[B,C,H,W] f32 (B=32, C=64, H=W=112, K=5):
    xp  = x ** p                        (p == 2.0 per channel)
    sf  = depthwise_conv(xp, k 5x5 uniform, VALID) edge-padded back to HxW
    out = w * xp / (sigma**p + sf) + b

Fast path (16-bit, channels sharded 8-way, all batches kept per core):
  - Host converts x to fp16; per-core slab [B, CPC, H, W] fp16.
  - SBUF layout per channel: H(112) on partitions, (b, w) flattened on the
    free dim (32 x 112 = 3584 elements).
  - xp = x*x -> bf16 (bf16 keeps f32 exponent range, so tiny x**2 values
    stay relative-accurate; fp16 would flush them).
  - Windowed scan (f32 state) computes the 5-tap W-window sums directly:
        state_s = (xp[s] + state) - xp[s-5]
    output fp16.  Segment-boundary columns w in {0,1,110,111} are wrong
    and get fixed on the recip output by 4 small strided copies.
  - One fp16 matmul per 448-wide chunk against a banded V matrix applies
    the 5-tap H-window sum and the H edge replication (PSUM f32).
  - R = w/(k*den_raw + sigma**p) in ONE ScalarE pass using the Reciprocal
    LUT (k/w and sigma**p/w folded into the activation's scale/bias
    immediates; requires channel-uniform params, which the reference
    generator guarantees - otherwise numpy fallback).
  - out = xp * R on DVE (bf16/fp16 operands -> 2x mode), DMA out bf16,
    host upcasts to f32.
  - Engine assignment of square/scan/fmult is tunable per channel between
    DVE / ScalarE / GpSimd to balance engine busy time (see ASSIGN).
"""

import numpy as np

B, C, H, W, KS = 32, 64, 112, 112, 5
NCORES = 8
CPC = C // NCORES          # channels per core
NSEG = B                   # free-dim segments per channel tile
F = NSEG * W               # free elements per channel tile = 3584
SCAN_N = F + 2             # scan length (need states up to s=F+1)
XP_F = 5 + F + 2           # xp tile: [0:5) zeros, [5:5+F) data, 2 zeros
NCHUNK = F // 448          # 448-wide matmul chunks per channel tile = 8

_CACHE = {}

# per-channel engine assignment for the 16-bit path (scan is DVE-only --
# the TensorTensorScanArith opcode is illegal on Pool):
#   square: A=scalar, D=vector, P=gpsimd ; fmult: D=vector, P=gpsimd
ASSIGN = {
    "h1": dict(sq="AAAAAPPP", fm="DDDDDPPP", fix="D", ms="D"),
    "hnp": dict(sq="AAAAAAAA", fm="DDDDDDDD", fix="D", ms="D"),
    "h2": dict(sq="AAPPPPPP", fm="DDDDDPPP", fix="D", ms="D"),
    "hpool": dict(sq="PPPPPPPP", fm="PPPPPPPP", fix="D", ms="D"),
    # post-DMA-fix balance: squares mostly Pool, fixes on ACT, memsets Pool
    "h4": dict(sq="APPPPPPP", fm="DDDPPPPP", fix="A", ms="P"),
    "h5": dict(sq="AAPPPPPP", fm="DDDDPPPP", fix="A", ms="P"),
    # balanced for real engine rates (Pool ~7.1us/pass, ACT 3.1, DVE-2x 1.9)
    "h6": dict(sq="AAAAADDP", fm="DDDDDDPP", fix="A", ms="D"),
    "hsq": dict(sq="DDDDDDDD", fm="DDDDDDDD", fix="D", ms="D"),
    "hsc": dict(sq="DDDDDDDD", fm="DDDDDDDD", fix="D", ms="D"),
    # h1 assignment with deeper sf/rt buffering
    "h8": dict(sq="AAAAAPPP", fm="DDDDDPPP", fix="D", ms="D", deep=True),
}


def prep_x(x16):
    """Per-core input slabs in [CPC, H, B, W] layout (contiguous h-rows)."""
    return [
        np.ascontiguousarray(
            x16[:, c * CPC:(c + 1) * CPC].transpose(1, 2, 0, 3))
        for c in range(NCORES)
    ]


def unprep_out(cores):
    """[CPC, H, B, W] per-core outputs -> [B, C, H, W]."""
    return np.concatenate(
        [np.asarray(o).transpose(2, 0, 1, 3) for o in cores], axis=1)


def _build16(reps: int, variant: str, scale: float, bias: float):
    import concourse.bacc as bacc
    import concourse.mybir as mybir
    import concourse.tile as tile
    import bass_rust as _bass_rust
    from concourse.hw_specs import get_activation_tables

    f32 = mybir.dt.float32
    f16 = mybir.dt.float16
    bf16 = mybir.dt.bfloat16
    Alu = mybir.AluOpType
    Act = mybir.ActivationFunctionType

    class _Bacc(bacc.Bacc):
        """Pin all activations to the reciprocal_and_small table set so a
        single ACT_TABLE_LOAD serves Reciprocal/Square/Copy."""

        def insert_act_table_loads(self):
            has_activation = any(
                isinstance(i, mybir.InstActivation)
                for b in self.main_func.blocks
                for i in b.instructions
            )
            if not has_activation:
                return
            ours = {Act.Reciprocal, Act.Square, Act.Copy}
            tables = []
            for name, fns in get_activation_tables(self.m.arch).items():
                if name != "reciprocal_and_small":
                    fns = fns - ours
                tables.append((name, fns))
            _bass_rust.insert_act_table_loads(self, tables)

    nc = _Bacc(
        "TRN2", target_bir_lowering=False, debug=False, enable_asserts=True,
        num_devices=NCORES,
    )

    dma_only = variant == "hdma"
    # probe variants: truncate the pipeline after N stages (timing only)
    probe_sq = variant == "hsq"
    probe_scan = variant == "hsc"
    out_dt = f16 if (dma_only or probe_scan) else bf16
    # host pre-transposes to [CPC, H, B, W] so each partition row (h) is
    # one contiguous 7168 B HBM run -> large DMA descriptors
    x_d = nc.dram_tensor("x", [CPC, H, B, W], f16, kind="ExternalInput")
    out_d = nc.dram_tensor("out", [CPC, H, B, W], out_dt,
                           kind="ExternalOutput")

    # Banded V matrix [h, h']: 1 iff clamp(h'-2,0,107) <= h <= clamp+4.
    v = np.zeros((H, H), np.float16)
    for hp in range(H):
        base = min(max(hp - 2, 0), H - KS)
        v[base:base + KS, hp] = 1.0
    vpos_d = nc.inline_tensor(v, name="vpos")

    asn = ASSIGN.get(variant, ASSIGN["h1"])

    def recip_act(out_ap, in_ap):
        eng = nc.scalar
        ins = [
            eng.lower_ap(in_ap),
            mybir.ImmediateValue(dtype=f32, value=float(bias)),
            mybir.ImmediateValue(dtype=f32, value=float(scale)),
            mybir.ImmediateValue(dtype=f32, value=0.0),
        ]
        return eng.add_instruction(
            mybir.InstActivation(
                name=nc.get_next_instruction_name(),
                func=Act.Reciprocal,
                ins=ins,
                outs=[eng.lower_ap(out_ap)],
            )
        )

    deep = bool(asn.get("deep"))
    nb = 3 if deep else 2
    with tile.TileContext(nc) as tc:
        with (
            tc.tile_pool(name="const", bufs=1) as const_pool,
            tc.tile_pool(name="xin", bufs=4 if deep else 3) as xin_pool,
            tc.tile_pool(name="sf", bufs=nb) as sf_pool,
            tc.tile_pool(name="rt", bufs=nb) as rt_pool,
            tc.tile_pool(name="outt", bufs=4 if deep else 3) as out_pool,
            tc.tile_pool(name="ps", bufs=2, space="PSUM") as ps_pool,
        ):
            vpos_sb = const_pool.tile([H, H], f16, tag="vpos")
            nc.sync.dma_start(vpos_sb[:], vpos_d[:])

            # xp double buffer with persistent zero pads (5 front, 2 back)
            xp_bufs = []
            if not dma_only:
                for tag in ("xpA", "xpB"):
                    xb = const_pool.tile([H, XP_F], bf16, tag=tag)
                    nc.vector.memset(xb[:, 0:5], 0.0)
                    nc.vector.memset(xb[:, 5 + F:XP_F], 0.0)
                    xp_bufs.append(xb)

            for it, ci in enumerate(
                    [c for _ in range(reps) for c in range(CPC)]):
                xt = xin_pool.tile([H, F], f16, tag="xt")
                nc.sync.dma_start(
                    xt[:].rearrange("p (b w) -> p b w", w=W), x_d[ci])
                if dma_only:
                    nc.sync.dma_start(
                        out_d[ci],
                        xt[:].rearrange("p (b w) -> p b w", w=W))
                    continue

                xpt = xp_bufs[it % 2]
                xpv = xpt[:, 5:5 + F]
                sq = asn["sq"][ci]
                if sq == "A":
                    nc.scalar.activation(xpv, xt[:], Act.Square)
                elif sq == "P":
                    nc.gpsimd.tensor_tensor(xpv, xt[:], xt[:], Alu.mult)
                else:
                    nc.vector.tensor_tensor(xpv, xt[:], xt[:], Alu.mult)

                if probe_sq:
                    # second square doubles DVE load to discriminate
                    # 1x vs 2x mode under the DMA floor
                    nc.vector.tensor_tensor(xpv, xt[:], xt[:], Alu.mult)
                    nc.sync.dma_start(
                        out_d[ci],
                        xpv.rearrange("p (b w) -> p b w", w=W))
                    continue

                # 5-tap W-window sums, fp16 out (f32 scan state); DVE-only
                sft = sf_pool.tile([H, SCAN_N], f16, tag="sft")
                nc.vector.tensor_tensor_scan(
                    sft[:], xpt[:, 5:5 + SCAN_N], xpt[:, 0:SCAN_N], 0.0,
                    Alu.add, Alu.subtract)
                if probe_scan:
                    nc.sync.dma_start(
                        out_d[ci],
                        sft[:, 0:F].rearrange("p (b w) -> p b w", w=W))
                    continue

                # H-window via banded matmul, 512-wide chunks (4 + 3);
                # recip PSUM->SBUF per group
                rt = rt_pool.tile([H, F], f16, tag="rt")
                for g, nq in ((0, 4), (1, 3)):
                    ps = ps_pool.tile([H, 2048], f32, tag="ps")
                    for q in range(nq):
                        c0 = g * 2048 + q * 512
                        nc.tensor.matmul(
                            ps[:, q * 512:(q + 1) * 512], vpos_sb[:],
                            sft[:, c0 + 2:c0 + 514], start=True, stop=True)
                    gsz = nq * 512
                    recip_act(rt[:, g * 2048:g * 2048 + gsz],
                              ps[:, 0:gsz])

                # fix W-edge columns (replicate pad) on the recip output
                rtv = rt[:].rearrange("p (s w) -> p s w", w=W)
                for dst, src in ((0, 2), (110, 109)):
                    src_b = rtv[:, :, src:src + 1].broadcast_to([H, NSEG, 2])
                    if asn.get("fix") == "A":
                        nc.scalar.copy(rtv[:, :, dst:dst + 2], src_b)
                    else:
                        nc.vector.tensor_copy(
                            out=rtv[:, :, dst:dst + 2], in_=src_b)

                # out = xp * R  (bf16 x fp16 -> bf16, DVE 2x)
                ot = out_pool.tile([H, F], bf16, tag="ot")
                fm_eng = nc.gpsimd if asn["fm"][ci] == "P" else nc.vector
                fm_eng.tensor_tensor(ot[:], xpv, rt[:], Alu.mult)

                nc.sync.dma_start(
                    out_d[ci],
                    ot[:].rearrange("p (b w) -> p b w", w=W))

    nc.compile()
    return nc


def _get_nc(variant: str, reps: int, scale: float, bias: float):
    key = ("nc", variant, reps, scale, bias)
    if key not in _CACHE:
        _CACHE[key] = _build16(reps, variant, scale, bias)
    return _CACHE[key]


def _kernel_fallback(x, sigma, pow_p, sum_kernel, weight, bias):
    """Pure-numpy fallback for inputs outside the fast-path preconditions."""
    xp = x.astype(np.float64) ** pow_p.reshape(1, -1, 1, 1)
    from numpy.lib.stride_tricks import sliding_window_view
    win = sliding_window_view(xp, (KS, KS), axis=(2, 3))
    sf = np.einsum("bchwij,cij->bchw", win, sum_kernel[:, 0].astype(np.float64))
    hk = KS // 2
    sf = np.pad(sf, ((0, 0), (0, 0), (hk, hk), (hk, hk)), mode="edge")
    den = (sigma.astype(np.float64) ** pow_p).reshape(1, -1, 1, 1) + sf
    out = weight.reshape(1, -1, 1, 1) * xp / den + bias.reshape(1, -1, 1, 1)
    return out.astype(np.float32)


def _fold_params(sigma, pow_p, sum_kernel, weight):
    """Return (scale, bias) floats if params are channel-uniform, else None."""
    kflat = sum_kernel.reshape(C, -1)
    if not np.all(kflat == kflat[0, 0]):
        return None
    spvals = (sigma.astype(np.float64) ** pow_p.astype(np.float64))
    if not (np.all(spvals == spvals[0]) and np.all(weight == weight[0])):
        return None
    w0 = float(weight[0])
    if w0 == 0.0 or not np.isfinite(spvals[0]):
        return None
    return float(kflat[0, 0] / w0), float(spvals[0] / w0)


def kernel(x, sigma, pow_p, sum_kernel, weight, bias, _variant="h1"):
    x = np.ascontiguousarray(np.asarray(x, dtype=np.float32))
    sigma = np.asarray(sigma, dtype=np.float32)
    pow_p = np.asarray(pow_p, dtype=np.float32)
    sum_kernel = np.asarray(sum_kernel, dtype=np.float32)
    weight = np.asarray(weight, dtype=np.float32)
    bias = np.asarray(bias, dtype=np.float32)

    folded = _fold_params(sigma, pow_p, sum_kernel, weight)
    if (x.shape != (B, C, H, W) or not np.all(pow_p == 2.0)
            or folded is None or np.any(x < 0.0)):
        return _kernel_fallback(x, sigma, pow_p, sum_kernel, weight, bias)
    scale, sbias = folded

    from concourse.bass_utils import run_bass_kernel_spmd

    x16 = x.astype(np.float16)
    in_maps = [{"x": slab} for slab in prep_x(x16)]

    nc = _get_nc(_variant, 1, scale, sbias)
    trace_kwargs = _CACHE.get("trace_kwargs") or {}
    res = run_bass_kernel_spmd(nc, in_maps, core_ids=list(range(NCORES)),
                               **trace_kwargs)
    _CACHE["last_results"] = res
    out = unprep_out(
        [res.results[i]["out"].astype(np.float32) for i in range(NCORES)])
    if np.any(bias != 0.0):
        out = out + bias.reshape(1, -1, 1, 1)
    return out
